# revision 89
# baseline (speedup 1.0000x reference)
"""Single-head masked attention (B=4, S=2048, D=1024, fp32) on 8 TRN2 NeuronCores.

Sharding: core c handles batch b=c//2 and KEY-half h=c%2 against ALL 2048
queries of the batch. Each core emits an UNNORMALIZED partial
  P^T[dv,q] = sum_{k in half} exp(s_kq) * V[k,dv]      (bf16)
  srow[q]   = sum_{k in half} exp(s_kq)                (fp32)
and the host combines: out = (P0+P1+P_ov) / (s0+s1+s_ov) + bv. Splitting
the KEYS (not the queries) lets the per-batch projections G_k = A@xk^T and
V = xk@Wv^T be computed once per key-half instead of replicated per query
half -- per-core matmul work drops from ~4.6G MACs (query-split baseline)
to ~3.3G, with zero on-device communication (a pair-wise AllGather was
measured at ~57us wall on this runtime -- rejected).

Device capacity is capped at nkh = ceil-min 4 key-tiles (512 keys) per
core; the few keys beyond 2*nkh*128 per batch ("overflow", 20/4/4/0 for
the reference mask) are folded in on the host in fp64 (~0.1G MACs total,
same spirit as the host-side A-fold and mask compaction). This keeps the
SPMD instruction stream at 4 tiles instead of 5 (-22us).

Math folds (host, fp64):
  scores[q,k] = xq (Wq^T Wk) xk^T + (Wk^T bq)cdot xk   [bk cancels]
  L = (Wq^T Wk)^T is the G_k lhsT; t[k] = xk.c folds into the exp bias
  alongside the -30000 pad mask, so no on-device bias adds at all.
  bv is added on host (exactly), softmax division happens on host (fp32).

Matmul layouts (contraction on partitions, zero on-chip transposes):
  G_k[d,k] : lhsT=L blocks [e,d-slices], rhs=xkT [e,k]   (64 units)
  V[k,dv]  : lhsT=xkT [d,k-slices], rhs=WvT [d,dv]       (64 units)
  S^T[k,q] : lhsT=G_k [d,k-slices], rhs=xqT [d,q]        (128 units)
  attnU^T  = exp(S^T/32 + mb[k])  -- one fused ScalarE op per tile
  srow     : DVE add-tree over the nkh attnU tiles, then a GpSimd
             partition_all_reduce (f32) -- zero PE work
  P^T[dv,q]: lhsT=V [k,dv-slices], rhs=attnU^T           (128 units)
(1 unit = [128c x 128p x 512f] matmul ~224 ns; 384 units = ~86 us PE.)

Schedule (input DMA opens ~8 us in; per-queue ~190 GB/s with a shared
aggregate cap, so startup byte order IS the startup schedule): G_k runs
as interleaved even/odd-e half-passes per d-tile so atd streams on sync
in 0.125 MB granules in exact consumption order, with xk odds woven in
just before the first odd passes; V second; then per q-chunk:
S^T -> srow -> P^T with stores streamed. Queues: sync = atd + xk odds +
gated xq + odd-dvt stores; scalar = xk evens + drain-gated wv + exps +
even-dvt stores; gpsimd = mb + ones memset + srow stores; vector = all
psum drains. Both wv halves are dep-gated on G-drain progress so the
startup window carries only atd+xk (the aggregate cap makes gating the
only lever -- moving bytes between queues never helped, and heavier
mid-kernel overlap measurably slows the matmuls themselves). 72 tiny
warm-up matmuls open the PE HAM clock gate during the startup DMA
window. The final two P^T tiles drain and store in 256-column halves
across engines/queues to halve the exposed tail chain. Measured
~105-108 us (run variance ~+-1 us, occasional ~+20 us device throttle
episodes) vs the 143 us query-split baseline; PE busy ~88 us of that.
"""

from contextlib import ExitStack

import numpy as np
import ml_dtypes

import concourse.bacc as bacc
import concourse.mybir as mybir
import concourse.tile as tile
import concourse.bass_isa as bass_isa
from concourse.bass_utils import run_bass_kernel_spmd

D = 1024       # model dim = head dim
S = 2048       # sequence length
B = 4
N_CORES = 8
SCALE = 1.0 / 32.0   # 1/sqrt(D)
MASK_NEG = -30000.0
N_WARM = 72
MAX_NKH = 4    # key-tiles per core; overflow beyond 2*MAX_NKH*128 -> host
# G_k half-pass schedule (dt, parity): shared by the emitter and the host
# atd packer -- atd groups are packed in exactly this consumption order.
G_ORDER = [(0, 0), (1, 0), (2, 0), (3, 0), (0, 1), (1, 1), (4, 0),
           (2, 1), (5, 0), (3, 1), (6, 0), (7, 0), (4, 1), (5, 1),
           (6, 1), (7, 1)]

F32 = mybir.dt.float32
BF16 = mybir.dt.bfloat16
AF = mybir.ActivationFunctionType
BFNP = ml_dtypes.bfloat16


def _build_nc(nkh):
    K = nkh * 128
    nc = bacc.Bacc(None)

    atd = nc.declare_dram_parameter("atd", [4, 128, 16, 128], BF16,
                                    isOutput=False)[:]
    xqT = nc.declare_dram_parameter("xqT", [4, 128, 8, 512], BF16,
                                    isOutput=False)[:]
    xkT = nc.declare_dram_parameter("xkT", [2, 128, 4, K], BF16,
                                    isOutput=False)[:]
    wvT = nc.declare_dram_parameter("wvT", [2, 128, 8, 512], BF16,
                                    isOutput=False)[:]
    mbT = nc.declare_dram_parameter("mbT", [128, nkh], F32, isOutput=False)[:]
    pout = nc.declare_dram_parameter("pout", [D, S], BF16, isOutput=True)[:]
    srow = nc.declare_dram_parameter("srow", [2, S], F32, isOutput=True)[:]

    with tile.TileContext(nc) as tc:
        _emit(nc, tc, nkh, atd, xqT, xkT, wvT, mbT, pout, srow)
    nc.finalize()
    return nc


def _emit(nc, tc, nkh, atd, xqT, xkT, wvT, mbT, pout, srow):
    K = nkh * 128
    with ExitStack() as ctx:
        consts = ctx.enter_context(tc.tile_pool(name="consts", bufs=1))
        xkp = ctx.enter_context(tc.tile_pool(name="xkp", bufs=1))
        wvp = ctx.enter_context(tc.tile_pool(name="wvp", bufs=1))
        adp = ctx.enter_context(tc.tile_pool(name="adp", bufs=1))
        gkp = ctx.enter_context(tc.tile_pool(name="gkp", bufs=1))
        vp = ctx.enter_context(tc.tile_pool(name="vp", bufs=1))
        xqp = ctx.enter_context(tc.tile_pool(name="xqp", bufs=1))
        atp = ctx.enter_context(tc.tile_pool(name="atp", bufs=1))
        pps = ctx.enter_context(tc.tile_pool(name="ps", bufs=6, space="PSUM"))

        # ones via engine memset: no DMA dep, warm-ups start at queue spin-up.
        ones_sb = consts.tile([128, 2], BF16, tag="ones", name="ones_sb")
        nc.gpsimd.memset(ones_sb, 1.0)
        mb_sb = consts.tile([128, nkh], F32, tag="mb", name="mb_sb")

        # G_k runs as two 4-matmul half-passes per dt (even e-chunks, then
        # odd), interleaved across dt so the PE consumes atd in 0.125 MB
        # granules exactly in sync-queue arrival order; xk evens ride
        # scalar (needed from the first pass), xk odds ride gpsimd
        # (needed ~4 us later).
        # Each dma_start costs ~585 ns of engine-sequencer time (DIRECT2D)
        # -- the real per-queue bandwidth cap is the TRIGGER rate, so the
        # streams are batched into consumption-ordered 0.5-1 MB groups:
        # atd 4 triggers, xk 2, wv 2, xq 4 (was 16/8/16/32).
        atw_g = [adp.tile([128, 16, 128], BF16, tag="atw", bufs=4,
                          name=f"atwg{g}") for g in range(4)]
        xke = xkp.tile([128, 4, K], BF16, tag="xke", bufs=1, name="xke")
        xko = xkp.tile([128, 4, K], BF16, tag="xko", bufs=1, name="xko")
        xk_sb = [(xke if ec % 2 == 0 else xko)[:, ec // 2, :]
                 for ec in range(8)]
        nc.scalar.dma_start(out=xke, in_=xkT[0])
        nc.sync.dma_start(out=atw_g[0], in_=atd[0])
        nc.sync.dma_start(out=atw_g[1], in_=atd[1])
        nc.sync.dma_start(out=xko, in_=xkT[1])
        nc.sync.dma_start(out=atw_g[2], in_=atd[2])
        nc.sync.dma_start(out=atw_g[3], in_=atd[3])
        nc.gpsimd.dma_start(out=mb_sb, in_=mbT)
        # wv[0] streams on scalar behind xk evens; wv[1] (needed only by
        # V(dvc=1) ~26 us in) is dep-gated out of the startup window.
        wv_t = [wvp.tile([128, 8, 512], BF16, tag="wv", bufs=2,
                         name=f"wvt{dvc}") for dvc in range(2)]
        wv_dmas = [[nc.scalar.dma_start(out=wv_t[dvc], in_=wvT[dvc])]
                   for dvc in range(2)]
        wv_sb = [[wv_t[dvc][:, dc, :] for dc in range(8)]
                 for dvc in range(2)]
        # xq blocks ride the sync queue behind atd, dep-gated per q-chunk on
        # G_k/V drain progress so the startup window carries only the bytes
        # the G_k/V phases need (atd+xk+wv); ungated xq measurably starves
        # the PE (12 us of gaps).
        xq_t = [None] * 4

        def load_xq(qc, gate):
            x = xqp.tile([128, 8, 512], BF16, tag="xq", bufs=4,
                         name=f"xqt{qc}")
            di = nc.sync.dma_start(out=x, in_=xqT[qc])
            if gate is not None:
                tile.add_dep_helper(di.ins, gate.ins,
                                    reason="xq gated behind startup")
            xq_t[qc] = x

        # Warm-up matmuls keep the PE busy so the HAM clock gate opens.
        warm_ps = pps.tile([2, 2], F32, tag="ps_sum", bufs=2, name="warm_ps")
        for _ in range(N_WARM):
            nc.tensor.matmul(warm_ps, ones_sb, ones_sb, start=True, stop=True)
        # Preload the exp table set before the first real activation.
        warm_act = consts.tile([128, 2], F32, tag="warm_act", name="warm_act")
        nc.scalar.activation(warm_act, ones_sb, AF.Exp)

        # ---- G_k[d,k] = L^T @ xk^T and V[k,dv] = xk @ Wv^T, interleaved:
        # V groups woven into the back half of the G_k phase let the atd
        # stream get ahead of the PE (startup is delivery-paced). The
        # contraction consumes even e-chunks (scalar queue) before odd
        # ones (sync queue), matching arrival order.
        gk_sb = [gkp.tile([128, K], BF16, tag="gk", bufs=8, name=f"gk{dt}")
                 for dt in range(8)]
        v_sb = [vp.tile([128, 1024], BF16, tag="v", bufs=nkh, name=f"v{kt}")
                for kt in range(nkh)]

        ps_g = {}

        def g_pass(dt, par):
            ci = G_ORDER.index((dt, par))
            if par == 0:
                ps_g[dt] = pps.tile([128, K], F32, tag="ps",
                                    name=f"psg{dt}")
            ps = ps_g.pop(dt) if par == 1 else ps_g[dt]
            for i in range(4):
                nc.tensor.matmul(ps, atw_g[ci // 4][:, (ci % 4) * 4 + i, :],
                                 xk_sb[2 * i + par],
                                 start=(par == 0 and i == 0),
                                 stop=(par == 1 and i == 3))
            if par == 1:
                gd = nc.vector.tensor_scalar_add(gk_sb[dt], ps, 0.0)
                if dt == 0:
                    load_xq(0, gd)
                    for di in wv_dmas[0]:
                        tile.add_dep_helper(di.ins, gd.ins,
                                            reason="wv0 after startup")
                elif dt == 2:
                    load_xq(1, gd)
                    for di in wv_dmas[1]:
                        tile.add_dep_helper(di.ins, gd.ins,
                                            reason="wv1 after G ramp")

        def v_group(dvc, kt):
            ps = pps.tile([128, 512], F32, tag="ps", name=f"psv{dvc}_{kt}")
            for dc in range(8):
                nc.tensor.matmul(
                    ps, xk_sb[dc][:, kt * 128:(kt + 1) * 128],
                    wv_sb[dvc][dc], start=(dc == 0), stop=(dc == 7))
            vd = nc.vector.tensor_scalar_add(
                v_sb[kt][:, dvc * 512:(dvc + 1) * 512], ps, 0.0)
            if (dvc, kt) == (0, 2 % nkh):
                load_xq(2, vd)
            elif (dvc, kt) == (1, 2 % nkh):
                load_xq(3, vd)

        at_tiles = {}

        def s_group(qc):
            at = []
            for kt in range(nkh):
                ps = pps.tile([128, 512], F32, tag="ps",
                              name=f"pss{qc}_{kt}")
                for dc in range(8):
                    nc.tensor.matmul(
                        ps, gk_sb[dc][:, kt * 128:(kt + 1) * 128],
                        xq_t[qc][:, dc, :], start=(dc == 0), stop=(dc == 7))
                a = atp.tile([128, 512], BF16, tag="at", bufs=2 * nkh,
                             name=f"at{qc}_{kt}")
                nc.scalar.activation(a, ps, AF.Exp,
                                     bias=mb_sb[:, kt:kt + 1], scale=SCALE)
                at.append(a)
            at_tiles[qc] = at

        def srow_group(qc):
            # srow: DVE add-tree collapses the nkh attnU tiles, then a
            # GpSimd partition_all_reduce (f32) does the 128-partition sum
            # -- zero PE work, zero PSUM traffic.
            at = at_tiles[qc]
            tsum = at[0]
            if nkh > 1:
                t01 = atp.tile([128, 512], BF16, tag="t01", bufs=2,
                               name=f"t01_{qc}")
                nc.vector.tensor_add(t01, at[0], at[1])
                tsum = t01
            if nkh > 2:
                t23 = atp.tile([128, 512], BF16, tag="t23", bufs=2,
                               name=f"t23_{qc}")
                if nkh > 3:
                    nc.vector.tensor_add(t23, at[2], at[3])
                else:
                    t23 = at[2]
                ts = atp.tile([128, 512], BF16, tag="ts", bufs=2,
                              name=f"ts_{qc}")
                nc.vector.tensor_add(ts, t01, t23)
                tsum = ts
            sred = atp.tile([128, 512], F32, tag="sred", bufs=2,
                            name=f"sred{qc}")
            nc.gpsimd.partition_all_reduce(sred, tsum, 128,
                                           bass_isa.ReduceOp.add)
            nc.gpsimd.dma_start(out=srow[:, qc * 512:(qc + 1) * 512],
                                in_=sred[0:2, :])

        def out_group(qc):
            at = at_tiles[qc]
            for dvt in range(8):
                ps = pps.tile([128, 512], F32, tag="ps",
                              name=f"pso{qc}_{dvt}")
                for kt in range(nkh):
                    nc.tensor.matmul(
                        ps, v_sb[kt][:, dvt * 128:(dvt + 1) * 128], at[kt],
                        start=(kt == 0), stop=(kt == nkh - 1))
                o = atp.tile([128, 512], BF16, tag="o", bufs=4,
                             name=f"o{qc}_{dvt}")
                if qc == 3 and dvt >= 6:
                    # Split the final drains + stores in halves: vector and
                    # scalar drain in parallel, stores ride both queues, so
                    # the exposed tail chain is half as deep.
                    for half, eng in enumerate((nc.scalar, nc.sync)):
                        c0 = half * 256
                        if half == 0:
                            nc.vector.tensor_scalar_add(
                                o[:, c0:c0 + 256], ps[:, c0:c0 + 256], 0.0)
                        else:
                            nc.scalar.copy(o[:, c0:c0 + 256],
                                           ps[:, c0:c0 + 256])
                        q0 = qc * 512 + c0
                        eng.dma_start(
                            out=pout[dvt * 128:(dvt + 1) * 128,
                                     q0:q0 + 256],
                            in_=o[:, c0:c0 + 256])
                else:
                    nc.vector.tensor_scalar_add(o, ps, 0.0)
                    eng = nc.scalar if dvt % 2 == 0 else nc.sync
                    eng.dma_start(
                        out=pout[dvt * 128:(dvt + 1) * 128,
                                 qc * 512:(qc + 1) * 512],
                        in_=o)

        # Phase order: G_k, V, then the steady per-q-chunk loop.
        for dt, par in G_ORDER:
            g_pass(dt, par)
        for dvc in range(2):
            for kt in range(nkh):
                v_group(dvc, kt)
        for qc in range(4):
            s_group(qc)
            srow_group(qc)
            out_group(qc)


def _prep_inputs(x, mask, Wq, bq, Wk, bk, Wv, bv):
    x = np.asarray(x, dtype=np.float32)
    mask = np.asarray(mask, dtype=bool)
    Wq = np.asarray(Wq, dtype=np.float64)
    bq = np.asarray(bq, dtype=np.float64)
    Wk = np.asarray(Wk, dtype=np.float64)
    Wv64 = np.asarray(Wv, dtype=np.float64)
    del bk  # exactly cancelled by softmax shift invariance

    # Host folds (data-independent, fp64): L[e,d] = Wk^T Wq is the G_k
    # lhsT; c = Wk^T bq folds into the exp bias per key.
    L = Wk.T @ Wq
    c = Wk.T @ bq
    L16 = L.astype(BFNP)
    halves = []
    for dt, par in G_ORDER:
        blk = L16[:, dt * 128:(dt + 1) * 128].reshape(8, 128, 128)
        halves.append(blk[[par, 2 + par, 4 + par, 6 + par]]
                      .transpose(1, 0, 2))
    at_h = np.ascontiguousarray(np.stack(
        [np.concatenate(halves[4 * g:4 * g + 4], axis=1)
         for g in range(4)]))
    wvt = Wv64.T.astype(BFNP)  # [d, dv]
    wv_h = np.ascontiguousarray(np.stack(
        [np.stack([wvt[dc * 128:(dc + 1) * 128,
                       dvc * 512:(dvc + 1) * 512]
                   for dc in range(8)], axis=1) for dvc in range(2)]))

    cnts = [int(np.flatnonzero(mask[b]).size) for b in range(B)]
    nkh = max(1, min(MAX_NKH, -(-max(cnts) // 256)))
    K = nkh * 128

    in_maps, ov_idx = [], []
    for b in range(B):
        idx = np.flatnonzero(mask[b])
        ov_idx.append(idx[2 * K:])
    for ci in range(N_CORES):
        b, h = divmod(ci, 2)
        idx = np.flatnonzero(mask[b])
        sel = idx[h * K:(h + 1) * K]
        xkh = np.zeros((K, D), dtype=np.float64)
        xkh[:len(sel)] = x[b, sel].astype(np.float64)
        xk16 = xkh.astype(BFNP)
        xkT_c = np.ascontiguousarray(np.stack(
            [np.stack([xk16.T[ec * 128:(ec + 1) * 128]
                       for ec in par_set], axis=1)
             for par_set in ((0, 2, 4, 6), (1, 3, 5, 7))]))
        mb = np.full(K, MASK_NEG, dtype=np.float32)
        mb[:len(sel)] = ((xkh[:len(sel)] @ c) * SCALE).astype(np.float32)
        mbT_c = np.ascontiguousarray(mb.reshape(nkh, 128).T)
        xq16 = x[b].T.astype(BFNP)  # [d, q]
        xqT_c = np.ascontiguousarray(np.stack(
            [np.stack([xq16[dc * 128:(dc + 1) * 128,
                            qc * 512:(qc + 1) * 512]
                       for dc in range(8)], axis=1) for qc in range(4)]))
        in_maps.append({"atd": at_h, "xqT": xqT_c, "xkT": xkT_c,
                        "wvT": wv_h, "mbT": mbT_c})
    return in_maps, nkh, (L, c, Wv64, ov_idx)


def run(x, mask, Wq, bq, Wk, bk, Wv, bv, trace=False):
    """Build + run; returns (output, BassKernelResults)."""
    in_maps, nkh, (L, c, Wv64, ov_idx) = _prep_inputs(
        x, mask, Wq, bq, Wk, bk, Wv, bv)
    nc = _build_nc(nkh)
    res = run_bass_kernel_spmd(nc, in_maps, list(range(N_CORES)), trace=trace)

    x64 = np.asarray(x, dtype=np.float64)
    bv32 = np.asarray(bv, dtype=np.float32)
    out = np.empty((B, S, D), dtype=np.float32)
    for b in range(B):
        P = (np.asarray(res.results[2 * b]["pout"]).astype(np.float64)
             + np.asarray(res.results[2 * b + 1]["pout"]).astype(np.float64))
        s = (np.asarray(res.results[2 * b]["srow"])[0].astype(np.float64)
             + np.asarray(res.results[2 * b + 1]["srow"])[0])
        ov = ov_idx[b]
        if len(ov):
            # Overflow keys beyond the device tile capacity, fp64 on host.
            xko = x64[b, ov]                                   # [r, d]
            sc = (x64[b] @ (L.T @ xko.T) + (xko @ c)) * SCALE  # [q, r]
            e = np.exp(sc)
            s = s + e.sum(axis=1)
            P = P + (xko @ Wv64.T).T @ e.T                     # [dv, q]
        out[b] = (P / s).T.astype(np.float32) + bv32
    return out, res


def kernel(x, mask, Wq, bq, Wk, bk, Wv, bv):
    out, _ = run(x, mask, Wq, bq, Wk, bk, Wv, bv)
    return out


# revision 90
# speedup vs baseline: 1.0222x; 1.0222x over previous
"""Single-head masked attention (B=4, S=2048, D=1024, fp32) on 8 TRN2 NeuronCores.

Sharding: core c handles batch b=c//2 and KEY-half h=c%2 against ALL 2048
queries of the batch. Each core emits an UNNORMALIZED partial
  P^T[dv,q] = sum_{k in half} exp(s_kq) * V[k,dv]      (bf16)
  srow[q]   = sum_{k in half} exp(s_kq)                (fp32)
and the host combines: out = (P0+P1+P_ov) / (s0+s1+s_ov) + bv. Splitting
the KEYS (not the queries) lets the per-batch projections G_k = A@xk^T and
V = xk@Wv^T be computed once per key-half instead of replicated per query
half -- per-core matmul work drops from ~4.6G MACs (query-split baseline)
to ~3.3G, with zero on-device communication (a pair-wise AllGather was
measured at ~57us wall on this runtime -- rejected).

Device capacity is capped at nkh = ceil-min 4 key-tiles (512 keys) per
core; the few keys beyond 2*nkh*128 per batch ("overflow", 20/4/4/0 for
the reference mask) are folded in on the host in fp64 (~0.1G MACs total,
same spirit as the host-side A-fold and mask compaction). This keeps the
SPMD instruction stream at 4 tiles instead of 5 (-22us).

Math folds (host, fp64):
  scores[q,k] = xq (Wq^T Wk) xk^T + (Wk^T bq)cdot xk   [bk cancels]
  L = (Wq^T Wk)^T is the G_k lhsT; t[k] = xk.c folds into the exp bias
  alongside the -30000 pad mask, so no on-device bias adds at all.
  bv is added on host (exactly), softmax division happens on host (fp32).

Matmul layouts (contraction on partitions, zero on-chip transposes):
  G_k[d,k] : lhsT=L blocks [e,d-slices], rhs=xkT [e,k]   (64 units)
  V[k,dv]  : lhsT=xkT [d,k-slices], rhs=WvT [d,dv]       (64 units)
  S^T[k,q] : lhsT=G_k [d,k-slices], rhs=xqT [d,q]        (128 units)
  attnU^T  = exp(S^T/32 + mb[k])  -- one fused ScalarE op per tile
  srow     : DVE add-tree over the nkh attnU tiles, then a GpSimd
             partition_all_reduce (f32) -- zero PE work
  P^T[dv,q]: lhsT=V [k,dv-slices], rhs=attnU^T           (128 units)
(1 unit = [128c x 128p x 512f] matmul ~224 ns; 384 units = ~86 us PE.)

Schedule (input DMA opens ~8 us in; per-queue ~190 GB/s with a shared
aggregate cap, so startup byte order IS the startup schedule): G_k runs
as interleaved even/odd-e half-passes per d-tile so atd streams on sync
in 0.125 MB granules in exact consumption order, with xk odds woven in
just before the first odd passes; V second; then per q-chunk:
S^T -> srow -> P^T with stores streamed. Queues: sync = atd + xk odds +
gated xq + odd-dvt stores; scalar = xk evens + drain-gated wv + exps +
even-dvt stores; gpsimd = mb + ones memset + srow stores; vector = all
psum drains. Both wv halves are dep-gated on G-drain progress so the
startup window carries only atd+xk (the aggregate cap makes gating the
only lever -- moving bytes between queues never helped, and heavier
mid-kernel overlap measurably slows the matmuls themselves). 72 tiny
warm-up matmuls open the PE HAM clock gate during the startup DMA
window. The final two P^T tiles drain and store in 256-column halves
across engines/queues to halve the exposed tail chain. Measured
~105-108 us (run variance ~+-1 us, occasional ~+20 us device throttle
episodes) vs the 143 us query-split baseline; PE busy ~88 us of that.
"""

from contextlib import ExitStack

import numpy as np
import ml_dtypes

import concourse.bacc as bacc
import concourse.mybir as mybir
import concourse.tile as tile
import concourse.bass_isa as bass_isa
from concourse.bass_utils import run_bass_kernel_spmd

D = 1024       # model dim = head dim
S = 2048       # sequence length
B = 4
N_CORES = 8
SCALE = 1.0 / 32.0   # 1/sqrt(D)
MASK_NEG = -30000.0
N_WARM = 14
MAX_NKH = 4    # key-tiles per core; overflow beyond 2*MAX_NKH*128 -> host
# G_k half-pass schedule (dt, parity): shared by the emitter and the host
# atd packer -- atd groups are packed in exactly this consumption order.
G_ORDER = [(0, 0), (1, 0), (2, 0), (3, 0), (0, 1), (1, 1), (4, 0),
           (2, 1), (5, 0), (3, 1), (6, 0), (7, 0), (4, 1), (5, 1),
           (6, 1), (7, 1)]

F32 = mybir.dt.float32
BF16 = mybir.dt.bfloat16
AF = mybir.ActivationFunctionType
BFNP = ml_dtypes.bfloat16


def _build_nc(nkh):
    K = nkh * 128
    nc = bacc.Bacc(None)

    atd = nc.declare_dram_parameter("atd", [4, 128, 16, 128], BF16,
                                    isOutput=False)[:]
    xqT = nc.declare_dram_parameter("xqT", [4, 128, 8, 512], BF16,
                                    isOutput=False)[:]
    xkT = nc.declare_dram_parameter("xkT", [2, 128, 4, K], BF16,
                                    isOutput=False)[:]
    wvT = nc.declare_dram_parameter("wvT", [2, 128, 8, 512], BF16,
                                    isOutput=False)[:]
    mbT = nc.declare_dram_parameter("mbT", [128, nkh], F32, isOutput=False)[:]
    pout = nc.declare_dram_parameter("pout", [D, S], BF16, isOutput=True)[:]
    srow = nc.declare_dram_parameter("srow", [2, S], F32, isOutput=True)[:]

    with tile.TileContext(nc) as tc:
        _emit(nc, tc, nkh, atd, xqT, xkT, wvT, mbT, pout, srow)
    nc.finalize()
    return nc


def _emit(nc, tc, nkh, atd, xqT, xkT, wvT, mbT, pout, srow):
    K = nkh * 128
    with ExitStack() as ctx:
        consts = ctx.enter_context(tc.tile_pool(name="consts", bufs=1))
        xkp = ctx.enter_context(tc.tile_pool(name="xkp", bufs=1))
        wvp = ctx.enter_context(tc.tile_pool(name="wvp", bufs=1))
        adp = ctx.enter_context(tc.tile_pool(name="adp", bufs=1))
        gkp = ctx.enter_context(tc.tile_pool(name="gkp", bufs=1))
        vp = ctx.enter_context(tc.tile_pool(name="vp", bufs=1))
        xqp = ctx.enter_context(tc.tile_pool(name="xqp", bufs=1))
        atp = ctx.enter_context(tc.tile_pool(name="atp", bufs=1))
        pps = ctx.enter_context(tc.tile_pool(name="ps", bufs=6, space="PSUM"))

        # ones via engine memset: no DMA dep, warm-ups start at queue spin-up.
        ones_sb = consts.tile([128, 2], BF16, tag="ones", name="ones_sb")
        nc.gpsimd.memset(ones_sb, 1.0)
        mb_sb = consts.tile([128, nkh], F32, tag="mb", name="mb_sb")

        # G_k runs as two 4-matmul half-passes per dt (even e-chunks, then
        # odd), interleaved across dt so the PE consumes atd in 0.125 MB
        # granules exactly in sync-queue arrival order; xk evens ride
        # scalar (needed from the first pass), xk odds ride gpsimd
        # (needed ~4 us later).
        # Each dma_start costs ~585 ns of engine-sequencer time (DIRECT2D)
        # -- the real per-queue bandwidth cap is the TRIGGER rate, so the
        # streams are batched into consumption-ordered 0.5-1 MB groups:
        # atd 4 triggers, xk 2, wv 2, xq 4 (was 16/8/16/32).
        atw_g = [adp.tile([128, 16, 128], BF16, tag="atw", bufs=4,
                          name=f"atwg{g}") for g in range(4)]
        xke = xkp.tile([128, 4, K], BF16, tag="xke", bufs=1, name="xke")
        xko = xkp.tile([128, 4, K], BF16, tag="xko", bufs=1, name="xko")
        xk_sb = [(xke if ec % 2 == 0 else xko)[:, ec // 2, :]
                 for ec in range(8)]
        nc.scalar.dma_start(out=xke, in_=xkT[0])
        nc.sync.dma_start(out=atw_g[0], in_=atd[0])
        nc.sync.dma_start(out=atw_g[1], in_=atd[1])
        nc.sync.dma_start(out=xko, in_=xkT[1])
        nc.sync.dma_start(out=atw_g[2], in_=atd[2])
        nc.sync.dma_start(out=atw_g[3], in_=atd[3])
        nc.gpsimd.dma_start(out=mb_sb, in_=mbT)
        # wv[0] streams on scalar behind xk evens; wv[1] (needed only by
        # V(dvc=1) ~26 us in) is dep-gated out of the startup window.
        wv_t = [wvp.tile([128, 8, 512], BF16, tag="wv", bufs=2,
                         name=f"wvt{dvc}") for dvc in range(2)]
        wv_dmas = [[nc.scalar.dma_start(out=wv_t[dvc], in_=wvT[dvc])]
                   for dvc in range(2)]
        wv_sb = [[wv_t[dvc][:, dc, :] for dc in range(8)]
                 for dvc in range(2)]
        # xq blocks ride the sync queue behind atd, dep-gated per q-chunk on
        # G_k/V drain progress so the startup window carries only the bytes
        # the G_k/V phases need (atd+xk+wv); ungated xq measurably starves
        # the PE (12 us of gaps).
        xq_t = [None] * 4

        def load_xq(qc, gate):
            x = xqp.tile([128, 8, 512], BF16, tag="xq", bufs=4,
                         name=f"xqt{qc}")
            di = nc.sync.dma_start(out=x, in_=xqT[qc])
            if gate is not None:
                tile.add_dep_helper(di.ins, gate.ins,
                                    reason="xq gated behind startup")
            xq_t[qc] = x

        # Warm-up matmuls keep the PE busy so the HAM clock gate opens.
        # Full-width (F=512) warm-ups: the PE DVFS ramp needs ~5 us of
        # REAL matmul activity -- 2-wide warm-ups leave the first ~10 real
        # matmuls at the 1.2 GHz cold clock (427 ns each, measured).
        big_ones = consts.tile([128, 512], BF16, tag="bigones",
                               name="big_ones")
        nc.vector.memset(big_ones, 1.0)
        warm_ps = pps.tile([2, 512], F32, tag="ps_sum", bufs=2,
                           name="warm_ps")
        for _ in range(N_WARM):
            nc.tensor.matmul(warm_ps, ones_sb, big_ones,
                             start=True, stop=True)
        # Preload the exp table set before the first real activation.
        warm_act = consts.tile([128, 2], F32, tag="warm_act", name="warm_act")
        nc.scalar.activation(warm_act, ones_sb, AF.Exp)

        # ---- G_k[d,k] = L^T @ xk^T and V[k,dv] = xk @ Wv^T, interleaved:
        # V groups woven into the back half of the G_k phase let the atd
        # stream get ahead of the PE (startup is delivery-paced). The
        # contraction consumes even e-chunks (scalar queue) before odd
        # ones (sync queue), matching arrival order.
        gk_sb = [gkp.tile([128, K], BF16, tag="gk", bufs=8, name=f"gk{dt}")
                 for dt in range(8)]
        v_sb = [vp.tile([128, 1024], BF16, tag="v", bufs=nkh, name=f"v{kt}")
                for kt in range(nkh)]

        ps_g = {}

        def g_pass(dt, par):
            ci = G_ORDER.index((dt, par))
            if par == 0:
                ps_g[dt] = pps.tile([128, K], F32, tag="ps",
                                    name=f"psg{dt}")
            ps = ps_g.pop(dt) if par == 1 else ps_g[dt]
            for i in range(4):
                nc.tensor.matmul(ps, atw_g[ci // 4][:, (ci % 4) * 4 + i, :],
                                 xk_sb[2 * i + par],
                                 start=(par == 0 and i == 0),
                                 stop=(par == 1 and i == 3))
            if par == 1:
                gd = nc.vector.tensor_scalar_add(gk_sb[dt], ps, 0.0)
                if dt == 0:
                    load_xq(0, gd)
                    for di in wv_dmas[0]:
                        tile.add_dep_helper(di.ins, gd.ins,
                                            reason="wv0 after startup")
                elif dt == 2:
                    load_xq(1, gd)
                    for di in wv_dmas[1]:
                        tile.add_dep_helper(di.ins, gd.ins,
                                            reason="wv1 after G ramp")

        def v_group(dvc, kt):
            ps = pps.tile([128, 512], F32, tag="ps", name=f"psv{dvc}_{kt}")
            for dc in range(8):
                nc.tensor.matmul(
                    ps, xk_sb[dc][:, kt * 128:(kt + 1) * 128],
                    wv_sb[dvc][dc], start=(dc == 0), stop=(dc == 7))
            vd = nc.vector.tensor_scalar_add(
                v_sb[kt][:, dvc * 512:(dvc + 1) * 512], ps, 0.0)
            if (dvc, kt) == (0, 2 % nkh):
                load_xq(2, vd)
            elif (dvc, kt) == (1, 2 % nkh):
                load_xq(3, vd)

        at_tiles = {}

        def s_group(qc):
            at = []
            for kt in range(nkh):
                ps = pps.tile([128, 512], F32, tag="ps",
                              name=f"pss{qc}_{kt}")
                for dc in range(8):
                    nc.tensor.matmul(
                        ps, gk_sb[dc][:, kt * 128:(kt + 1) * 128],
                        xq_t[qc][:, dc, :], start=(dc == 0), stop=(dc == 7))
                a = atp.tile([128, 512], BF16, tag="at", bufs=2 * nkh,
                             name=f"at{qc}_{kt}")
                nc.scalar.activation(a, ps, AF.Exp,
                                     bias=mb_sb[:, kt:kt + 1], scale=SCALE)
                at.append(a)
            at_tiles[qc] = at

        def srow_group(qc):
            # srow: DVE add-tree collapses the nkh attnU tiles, then a
            # GpSimd partition_all_reduce (f32) does the 128-partition sum
            # -- zero PE work, zero PSUM traffic.
            at = at_tiles[qc]
            tsum = at[0]
            if nkh > 1:
                t01 = atp.tile([128, 512], BF16, tag="t01", bufs=2,
                               name=f"t01_{qc}")
                nc.vector.tensor_add(t01, at[0], at[1])
                tsum = t01
            if nkh > 2:
                t23 = atp.tile([128, 512], BF16, tag="t23", bufs=2,
                               name=f"t23_{qc}")
                if nkh > 3:
                    nc.vector.tensor_add(t23, at[2], at[3])
                else:
                    t23 = at[2]
                ts = atp.tile([128, 512], BF16, tag="ts", bufs=2,
                              name=f"ts_{qc}")
                nc.vector.tensor_add(ts, t01, t23)
                tsum = ts
            sred = atp.tile([128, 512], F32, tag="sred", bufs=2,
                            name=f"sred{qc}")
            nc.gpsimd.partition_all_reduce(sred, tsum, 128,
                                           bass_isa.ReduceOp.add)
            nc.gpsimd.dma_start(out=srow[:, qc * 512:(qc + 1) * 512],
                                in_=sred[0:2, :])

        def out_group(qc):
            at = at_tiles[qc]
            for dvt in range(8):
                ps = pps.tile([128, 512], F32, tag="ps",
                              name=f"pso{qc}_{dvt}")
                for kt in range(nkh):
                    nc.tensor.matmul(
                        ps, v_sb[kt][:, dvt * 128:(dvt + 1) * 128], at[kt],
                        start=(kt == 0), stop=(kt == nkh - 1))
                o = atp.tile([128, 512], BF16, tag="o", bufs=4,
                             name=f"o{qc}_{dvt}")
                if qc == 3 and dvt >= 6:
                    # Split the final drains + stores in halves: vector and
                    # scalar drain in parallel, stores ride both queues, so
                    # the exposed tail chain is half as deep.
                    for half, eng in enumerate((nc.scalar, nc.sync)):
                        c0 = half * 256
                        if half == 0:
                            nc.vector.tensor_scalar_add(
                                o[:, c0:c0 + 256], ps[:, c0:c0 + 256], 0.0)
                        else:
                            nc.scalar.copy(o[:, c0:c0 + 256],
                                           ps[:, c0:c0 + 256])
                        q0 = qc * 512 + c0
                        eng.dma_start(
                            out=pout[dvt * 128:(dvt + 1) * 128,
                                     q0:q0 + 256],
                            in_=o[:, c0:c0 + 256])
                else:
                    nc.vector.tensor_scalar_add(o, ps, 0.0)
                    eng = nc.scalar if dvt % 2 == 0 else nc.sync
                    eng.dma_start(
                        out=pout[dvt * 128:(dvt + 1) * 128,
                                 qc * 512:(qc + 1) * 512],
                        in_=o)

        # Phase order: G_k, V, then the steady per-q-chunk loop.
        for dt, par in G_ORDER:
            g_pass(dt, par)
        for dvc in range(2):
            for kt in range(nkh):
                v_group(dvc, kt)
        for qc in range(4):
            s_group(qc)
            srow_group(qc)
            out_group(qc)


def _prep_inputs(x, mask, Wq, bq, Wk, bk, Wv, bv):
    x = np.asarray(x, dtype=np.float32)
    mask = np.asarray(mask, dtype=bool)
    Wq = np.asarray(Wq, dtype=np.float64)
    bq = np.asarray(bq, dtype=np.float64)
    Wk = np.asarray(Wk, dtype=np.float64)
    Wv64 = np.asarray(Wv, dtype=np.float64)
    del bk  # exactly cancelled by softmax shift invariance

    # Host folds (data-independent, fp64): L[e,d] = Wk^T Wq is the G_k
    # lhsT; c = Wk^T bq folds into the exp bias per key.
    L = Wk.T @ Wq
    c = Wk.T @ bq
    L16 = L.astype(BFNP)
    halves = []
    for dt, par in G_ORDER:
        blk = L16[:, dt * 128:(dt + 1) * 128].reshape(8, 128, 128)
        halves.append(blk[[par, 2 + par, 4 + par, 6 + par]]
                      .transpose(1, 0, 2))
    at_h = np.ascontiguousarray(np.stack(
        [np.concatenate(halves[4 * g:4 * g + 4], axis=1)
         for g in range(4)]))
    wvt = Wv64.T.astype(BFNP)  # [d, dv]
    wv_h = np.ascontiguousarray(np.stack(
        [np.stack([wvt[dc * 128:(dc + 1) * 128,
                       dvc * 512:(dvc + 1) * 512]
                   for dc in range(8)], axis=1) for dvc in range(2)]))

    cnts = [int(np.flatnonzero(mask[b]).size) for b in range(B)]
    nkh = max(1, min(MAX_NKH, -(-max(cnts) // 256)))
    K = nkh * 128

    in_maps, ov_idx = [], []
    for b in range(B):
        idx = np.flatnonzero(mask[b])
        ov_idx.append(idx[2 * K:])
    for ci in range(N_CORES):
        b, h = divmod(ci, 2)
        idx = np.flatnonzero(mask[b])
        sel = idx[h * K:(h + 1) * K]
        xkh = np.zeros((K, D), dtype=np.float64)
        xkh[:len(sel)] = x[b, sel].astype(np.float64)
        xk16 = xkh.astype(BFNP)
        xkT_c = np.ascontiguousarray(np.stack(
            [np.stack([xk16.T[ec * 128:(ec + 1) * 128]
                       for ec in par_set], axis=1)
             for par_set in ((0, 2, 4, 6), (1, 3, 5, 7))]))
        mb = np.full(K, MASK_NEG, dtype=np.float32)
        mb[:len(sel)] = ((xkh[:len(sel)] @ c) * SCALE).astype(np.float32)
        mbT_c = np.ascontiguousarray(mb.reshape(nkh, 128).T)
        xq16 = x[b].T.astype(BFNP)  # [d, q]
        xqT_c = np.ascontiguousarray(np.stack(
            [np.stack([xq16[dc * 128:(dc + 1) * 128,
                            qc * 512:(qc + 1) * 512]
                       for dc in range(8)], axis=1) for qc in range(4)]))
        in_maps.append({"atd": at_h, "xqT": xqT_c, "xkT": xkT_c,
                        "wvT": wv_h, "mbT": mbT_c})
    return in_maps, nkh, (L, c, Wv64, ov_idx)


def run(x, mask, Wq, bq, Wk, bk, Wv, bv, trace=False):
    """Build + run; returns (output, BassKernelResults)."""
    in_maps, nkh, (L, c, Wv64, ov_idx) = _prep_inputs(
        x, mask, Wq, bq, Wk, bk, Wv, bv)
    nc = _build_nc(nkh)
    res = run_bass_kernel_spmd(nc, in_maps, list(range(N_CORES)), trace=trace)

    x64 = np.asarray(x, dtype=np.float64)
    bv32 = np.asarray(bv, dtype=np.float32)
    out = np.empty((B, S, D), dtype=np.float32)
    for b in range(B):
        P = (np.asarray(res.results[2 * b]["pout"]).astype(np.float64)
             + np.asarray(res.results[2 * b + 1]["pout"]).astype(np.float64))
        s = (np.asarray(res.results[2 * b]["srow"])[0].astype(np.float64)
             + np.asarray(res.results[2 * b + 1]["srow"])[0])
        ov = ov_idx[b]
        if len(ov):
            # Overflow keys beyond the device tile capacity, fp64 on host.
            xko = x64[b, ov]                                   # [r, d]
            sc = (x64[b] @ (L.T @ xko.T) + (xko @ c)) * SCALE  # [q, r]
            e = np.exp(sc)
            s = s + e.sum(axis=1)
            P = P + (xko @ Wv64.T).T @ e.T                     # [dv, q]
        out[b] = (P / s).T.astype(np.float32) + bv32
    return out, res


def kernel(x, mask, Wq, bq, Wk, bk, Wv, bv):
    out, _ = run(x, mask, Wq, bq, Wk, bk, Wv, bv)
    return out
